# revision 10
# baseline (speedup 1.0000x reference)
"""Trainium2 Bass kernel for CausalSelfAttention (GQA + per-head RMS norm + RoPE).

Sharding: 8 cores = batch(2) x kv-head-group(4). Each core computes, for its
(b, g): qkv projection (its 4 rep q heads + 1 kv head), per-head RMS norm,
RoPE, causal attention, and a partial output projection (its 512 rows of
w_proj). Host sums the 4 partial projections per batch element.

Math notes:
  - Token-level rms_norm(x) commutes out of q/k (they are re-normalized per
    head, and rms_norm is scale-invariant), so only v is scaled by the
    per-token rstd(x). This avoids normalizing x before the qkv matmul and
    lets the qkv matmul run on raw x^T (built on-chip via PE transposes).
  - Scores are computed transposed (scoresT[s, t]) so that exp(scoresT) is
    directly the rhs of the attn@v matmul (contraction over s on partitions)
    and the attention output lands as aoT[d, t], which is exactly the lhsT
    the output projection needs. No attention-matrix transposes.
  - Softmax runs without max subtraction: q, k are unit-RMS (|q| <= 1.5*
    sqrt(128)) so scores*scale is bounded by ~17 and exp stays in fp32 range.
  - The softmax denominator is computed by a ones-vector matmul accumulated
    over s-tiles in PSUM, so the exp tiles are only ever consumed by the PE.
  - q_gain and the softmax 1/sqrt(HD) are folded into the per-head rstd and
    the exp scale respectively.
  - Matmul operands are stored as float32r (rounded at the producing
    instruction) for 4x PE throughput vs float32; KERNEL_MMDT=f32 falls back.
"""

import os

import numpy as np

from concourse import bacc, bass, mybir
from concourse import tile
from concourse.bass_utils import run_bass_kernel_spmd

# Problem shape (hardcoded per contract)
B, T, C = 2, 2048, 2048
N_HEADS, N_KV = 16, 4
HD = C // N_HEADS            # 128
REP = N_HEADS // N_KV        # 4
KV_DIM = N_KV * HD           # 512
P = 128
TT = T // P                  # 16 token tiles
KT = C // P                  # 16 contraction tiles
JQ = REP * HD                # 512 local q cols
JTOT = JQ + 2 * HD           # 768 local qkv cols
TCW = 512                    # attention t-chunk width
NTC = T // TCW               # 4
EPS = 1.1920929e-07
SCALE = 1.0 / float(np.sqrt(HD))

F32 = mybir.dt.float32
AF = mybir.ActivationFunctionType


def _emit(nc, mm_f32r=True):
    # MDT: dtype of every matmul operand tile. float32r tiles are written
    # (rounded) by the producing DVE/ACT/DMA instruction and only ever read
    # by the PE.
    MDT = mybir.dt.float32r if mm_f32r else F32

    x_d = nc.dram_tensor("xb", [T, C], F32, kind="ExternalInput")
    wqkv_d = nc.dram_tensor("wqkv", [C, JTOT], MDT, kind="ExternalInput")
    wproj_d = nc.dram_tensor("wproj", [JQ, C], MDT, kind="ExternalInput")
    gain_d = nc.dram_tensor("gain", [1, REP], F32, kind="ExternalInput")
    cos_d = nc.dram_tensor("costab", [T, HD], F32, kind="ExternalInput")
    sin_d = nc.dram_tensor("sintab", [T, HD], F32, kind="ExternalInput")  # [:, :64] = -sin
    mask_d = nc.dram_tensor("maskdiag", [4, P, TCW], F32, kind="ExternalInput")
    id_d = nc.dram_tensor("ident", [P, P], F32, kind="ExternalInput")
    y_d = nc.dram_tensor("y", [T, C], F32, kind="ExternalOutput")

    with tile.TileContext(nc) as tc:
        with tc.tile_pool(name="persist", bufs=1) as pp, \
             tc.tile_pool(name="psum", bufs=1, space="PSUM") as psp:
            # Long-lived activations (matmul operands -> MDT)
            qT = [pp.tile([P, T], MDT, name=f"qT{h}", tag=f"qT{h}") for h in range(REP)]
            kTt = pp.tile([P, T], MDT, name="kTt", tag="kTt")
            vN = pp.tile([P, TT, HD], MDT, name="vN", tag="vN")

            # ---------------- Phase 1: qkv + norms + rope + transposes -------
            with tc.tile_pool(name="ph1", bufs=1) as p1:
                wqkv_sb = p1.tile([P, KT, JTOT], MDT, name="wqkv_sb", tag="wqkv")
                nc.sync.dma_start(
                    out=wqkv_sb,
                    in_=wqkv_d.ap().rearrange("(kt p) j -> p kt j", p=P))
                cos_sb = p1.tile([P, TT, HD], F32, name="cos_sb", tag="cos")
                nc.sync.dma_start(
                    out=cos_sb, in_=cos_d.ap().rearrange("(tt p) d -> p tt d", p=P))
                sin_sb = p1.tile([P, TT, HD], F32, name="sin_sb", tag="sin")
                nc.sync.dma_start(
                    out=sin_sb, in_=sin_d.ap().rearrange("(tt p) d -> p tt d", p=P))
                id_sb = p1.tile([P, P], F32, name="id_sb", tag="ident")
                nc.sync.dma_start(out=id_sb, in_=id_d.ap())
                eps_t = p1.tile([P, 1], F32, name="eps_t", tag="eps")
                nc.vector.memset(eps_t, EPS)

                # broadcast gain [1,4] -> [128,4] via 0-stride DMA replication
                gainb = p1.tile([P, REP], F32, name="gainb", tag="gainb")
                nc.sync.dma_start(out=gainb,
                                  in_=gain_d.ap()[0].partition_broadcast(P))

                for tt in range(TT):
                    x_t = p1.tile([P, C], F32, name=f"x_{tt}", tag="x", bufs=2)
                    nc.sync.dma_start(out=x_t, in_=x_d.ap()[tt * P:(tt + 1) * P, :])

                    # token rstd (for v): 1/sqrt(mean(x^2)+eps)
                    sx4 = p1.tile([P, 4], F32, name=f"sx4_{tt}", tag="sx4", bufs=2)
                    for i in range(4):
                        scr = p1.tile([P, 512], F32, name=f"scrx_{tt}_{i}", tag="scr", bufs=2)
                        nc.scalar.activation(scr, x_t[:, i * 512:(i + 1) * 512],
                                             AF.Square, accum_out=sx4[:, i:i + 1])
                    ssx = p1.tile([P, 1], F32, name=f"ssx_{tt}", tag="ssx", bufs=2)
                    nc.vector.reduce_sum(ssx, sx4, axis=mybir.AxisListType.X)
                    srx = p1.tile([P, 1], F32, name=f"srx_{tt}", tag="srx", bufs=2)
                    nc.scalar.activation(srx, ssx, AF.Sqrt, scale=1.0 / C, bias=eps_t)
                    rstdx = p1.tile([P, 1], F32, name=f"rstdx_{tt}", tag="rstdx", bufs=2)
                    nc.vector.reciprocal(rstdx, srx)

                    # transpose raw x tile -> xT blocks (lhsT for qkv matmul)
                    xTt = p1.tile([P, C], MDT, name=f"xT_{tt}", tag="xT", bufs=3)
                    for grp in range(4):
                        tp = psp.tile([P, 512], F32, name=f"tp_{tt}_{grp}", tag="mm", bufs=3)
                        for j in range(4):
                            kt = grp * 4 + j
                            nc.tensor.transpose(tp[:, j * P:(j + 1) * P],
                                                x_t[:, kt * P:(kt + 1) * P], id_sb)
                        nc.vector.tensor_copy(xTt[:, grp * 512:(grp + 1) * 512], tp)

                    # qkv matmuls: q_ps [P, 512], kv_ps [P, 256]
                    q_ps = psp.tile([P, JQ], F32, name=f"qps_{tt}", tag="mm", bufs=3)
                    kv_ps = psp.tile([P, 2 * HD], F32, name=f"kvps_{tt}", tag="small", bufs=2)
                    for kt in range(KT):
                        lb = xTt[:, kt * P:(kt + 1) * P]
                        nc.tensor.matmul(q_ps, lb, wqkv_sb[:, kt, 0:JQ],
                                         start=(kt == 0), stop=(kt == KT - 1))
                        nc.tensor.matmul(kv_ps, lb, wqkv_sb[:, kt, JQ:JTOT],
                                         start=(kt == 0), stop=(kt == KT - 1))

                    # ---- q: per-head rms norm (x gain) + rope ----
                    ssq4 = p1.tile([P, REP], F32, name=f"ssq4_{tt}", tag="ssq4", bufs=2)
                    for h in range(REP):
                        scr = p1.tile([P, 512], F32, name=f"scrq_{tt}_{h}", tag="scr", bufs=2)
                        nc.scalar.activation(scr[:, :HD], q_ps[:, h * HD:(h + 1) * HD],
                                             AF.Square, accum_out=ssq4[:, h:h + 1])
                    srq = p1.tile([P, REP], F32, name=f"srq_{tt}", tag="srq", bufs=2)
                    nc.scalar.activation(srq, ssq4, AF.Sqrt, scale=1.0 / HD, bias=eps_t)
                    rstdq = p1.tile([P, REP], F32, name=f"rstdq_{tt}", tag="rstdq", bufs=2)
                    nc.vector.reciprocal(rstdq, srq)
                    rstdqg = p1.tile([P, REP], F32, name=f"rstdqg_{tt}", tag="rstdqg", bufs=2)
                    nc.vector.tensor_mul(rstdqg, rstdq, gainb)

                    qn_t = p1.tile([P, JQ], F32, name=f"qn_{tt}", tag="qn", bufs=2)
                    for h in range(REP):
                        nc.vector.tensor_scalar_mul(qn_t[:, h * HD:(h + 1) * HD],
                                                    q_ps[:, h * HD:(h + 1) * HD],
                                                    rstdqg[:, h:h + 1])
                    qn3 = qn_t.rearrange("p (h d) -> p h d", h=REP)
                    cos_t = cos_sb[:, tt, :]
                    sin_t = sin_sb[:, tt, :]
                    qf_t = p1.tile([P, JQ], F32, name=f"qf_{tt}", tag="qf", bufs=2)
                    qf3 = qf_t.rearrange("p (h d) -> p h d", h=REP)
                    qB_t = p1.tile([P, JQ], F32, name=f"qB_{tt}", tag="qB", bufs=2)
                    qB3 = qB_t.rearrange("p (h d) -> p h d", h=REP)
                    H2 = HD // 2
                    nc.vector.tensor_mul(qf3, qn3,
                                         cos_t[:, None, :].broadcast_to([P, REP, HD]))
                    nc.vector.tensor_mul(qB3[:, :, 0:H2], qn3[:, :, H2:HD],
                                         sin_t[:, None, 0:H2].broadcast_to([P, REP, H2]))
                    nc.vector.tensor_mul(qB3[:, :, H2:HD], qn3[:, :, 0:H2],
                                         sin_t[:, None, H2:HD].broadcast_to([P, REP, H2]))
                    nc.vector.tensor_add(qf3, qf3, qB3)

                    # ---- k: rms norm + rope ----
                    ssk = p1.tile([P, 1], F32, name=f"ssk_{tt}", tag="ssk", bufs=2)
                    scrk = p1.tile([P, 512], F32, name=f"scrk_{tt}", tag="scr", bufs=2)
                    nc.scalar.activation(scrk[:, :HD], kv_ps[:, 0:HD], AF.Square,
                                         accum_out=ssk)
                    srk = p1.tile([P, 1], F32, name=f"srk_{tt}", tag="srk", bufs=2)
                    nc.scalar.activation(srk, ssk, AF.Sqrt, scale=1.0 / HD, bias=eps_t)
                    rstdk = p1.tile([P, 1], F32, name=f"rstdk_{tt}", tag="rstdk", bufs=2)
                    nc.vector.reciprocal(rstdk, srk)
                    kn_t = p1.tile([P, HD], F32, name=f"kn_{tt}", tag="kn", bufs=2)
                    nc.vector.tensor_scalar_mul(kn_t, kv_ps[:, 0:HD], rstdk)
                    kf_t = p1.tile([P, HD], F32, name=f"kf_{tt}", tag="kf", bufs=2)
                    kB_t = p1.tile([P, HD], F32, name=f"kB_{tt}", tag="kB", bufs=2)
                    nc.vector.tensor_mul(kf_t, kn_t, cos_t)
                    nc.vector.tensor_mul(kB_t[:, 0:H2], kn_t[:, H2:HD], sin_t[:, 0:H2])
                    nc.vector.tensor_mul(kB_t[:, H2:HD], kn_t[:, 0:H2], sin_t[:, H2:HD])
                    nc.vector.tensor_add(kf_t, kf_t, kB_t)

                    # ---- v: scale rows by token rstd ----
                    nc.vector.tensor_scalar_mul(vN[:, tt, :], kv_ps[:, HD:2 * HD], rstdx)

                    # ---- transpose q heads + k into [d, t] layout ----
                    tq = psp.tile([P, 512], F32, name=f"tq_{tt}", tag="mm", bufs=3)
                    for h in range(REP):
                        nc.tensor.transpose(tq[:, h * P:(h + 1) * P],
                                            qf_t[:, h * P:(h + 1) * P], id_sb)
                    for h in range(REP):
                        nc.vector.tensor_copy(qT[h][:, tt * P:(tt + 1) * P],
                                              tq[:, h * P:(h + 1) * P])
                    tk = psp.tile([P, HD], F32, name=f"tk_{tt}", tag="small", bufs=2)
                    nc.tensor.transpose(tk, kf_t, id_sb)
                    nc.vector.tensor_copy(kTt[:, tt * P:(tt + 1) * P], tk)

            # ---------------- Phase 2+3: attention + projection --------------
            with tc.tile_pool(name="ph23", bufs=1) as p2:
                wproj_sb = p2.tile([P, REP, C], MDT, name="wproj_sb", tag="wproj")
                nc.sync.dma_start(
                    out=wproj_sb,
                    in_=wproj_d.ap().rearrange("(h p) c -> p h c", p=P))
                mask_sb = p2.tile([P, 4, TCW], F32, name="mask_sb", tag="mask")
                nc.sync.dma_start(
                    out=mask_sb, in_=mask_d.ap().rearrange("v p t -> p v t"))
                ones_f = p2.tile([P, P], F32, name="ones_f", tag="ones_f")
                nc.vector.memset(ones_f, 1.0)
                ones_col = p2.tile([P, 1], MDT, name="ones_col", tag="ones_col")
                nc.vector.tensor_copy(ones_col, ones_f[:, 0:1])
                ones_row = p2.tile([1, P], MDT, name="ones_row", tag="ones_row")
                nc.vector.tensor_copy(ones_row, ones_f[0:1, :])

                for tci in range(NTC):
                    ao = []
                    for h in range(REP):
                        o_ps = psp.tile([P, TCW], F32, name=f"ops_{tci}_{h}",
                                        tag="acc", bufs=2)
                        ds_ps = psp.tile([1, TCW], F32, name=f"ds_{tci}_{h}",
                                         tag="small", bufs=2)
                        nst = 4 * (tci + 1)
                        for st in range(nst):
                            sc = psp.tile([P, TCW], F32, name=f"sc_{tci}_{h}_{st}",
                                          tag="mm", bufs=3)
                            nc.tensor.matmul(sc, kTt[:, st * P:(st + 1) * P],
                                             qT[h][:, tci * TCW:(tci + 1) * TCW],
                                             start=True, stop=True)
                            et = p2.tile([P, TCW], MDT, name=f"et_{tci}_{h}_{st}",
                                         tag="et", bufs=6)
                            dv = st - 4 * tci
                            if dv >= 0:
                                er = p2.tile([P, TCW], F32, name=f"er_{tci}_{h}_{st}",
                                             tag="er", bufs=2)
                                nc.scalar.activation(er, sc, AF.Exp, scale=SCALE)
                                nc.vector.tensor_mul(et, er, mask_sb[:, dv, :])
                            else:
                                nc.scalar.activation(et, sc, AF.Exp, scale=SCALE)
                            # denominator: ones-matmul accumulated over s-tiles
                            nc.tensor.matmul(ds_ps, ones_col, et,
                                             start=(st == 0), stop=(st == nst - 1))
                            nc.tensor.matmul(o_ps, vN[:, st, :], et,
                                             start=(st == 0), stop=(st == nst - 1))
                        rcp = p2.tile([1, TCW], MDT, name=f"rcp_{tci}_{h}",
                                      tag="rcp", bufs=2)
                        with nc.allow_low_precision(reason="fp32r matmul operand"):
                            nc.vector.reciprocal(rcp, ds_ps)
                        rb_ps = psp.tile([P, TCW], F32, name=f"rb_{tci}_{h}",
                                         tag="mm", bufs=3)
                        nc.tensor.matmul(rb_ps, ones_row, rcp, start=True, stop=True)
                        rb = p2.tile([P, TCW], F32, name=f"rbs_{tci}_{h}",
                                     tag="rb", bufs=2)
                        nc.vector.tensor_copy(rb, rb_ps)
                        aot = p2.tile([P, TCW], MDT, name=f"ao_{tci}_{h}",
                                      tag="ao", bufs=8)
                        nc.vector.tensor_mul(aot, o_ps, rb)
                        ao.append(aot)

                    # projection for this t-chunk
                    for ttl in range(4):
                        yt = p2.tile([P, C], F32, name=f"y_{tci}_{ttl}", tag="y", bufs=2)
                        for ncs in range(4):
                            y_ps = psp.tile([P, 512], F32, name=f"yps_{tci}_{ttl}_{ncs}",
                                            tag="mm", bufs=3)
                            for h in range(REP):
                                nc.tensor.matmul(
                                    y_ps, ao[h][:, ttl * P:(ttl + 1) * P],
                                    wproj_sb[:, h, ncs * 512:(ncs + 1) * 512],
                                    start=(h == 0), stop=(h == REP - 1))
                            nc.vector.tensor_copy(yt[:, ncs * 512:(ncs + 1) * 512], y_ps)
                        row = (tci * 4 + ttl) * P
                        nc.sync.dma_start(out=y_d.ap()[row:row + P, :], in_=yt)

    return nc


_NC_CACHE = {}
LAST_RESULT = None


def _get_nc():
    key = os.environ.get("KERNEL_MMDT", "f32r")
    if key not in _NC_CACHE:
        nc = bacc.Bacc("TRN2", target_bir_lowering=False, debug=False)
        _emit(nc, mm_f32r=(key != "f32"))
        nc.compile()
        _NC_CACHE[key] = nc
    return _NC_CACHE[key]


def _host_tables():
    inv_freq = 1.0 / (10000.0 ** (np.arange(0, HD, 2, dtype=np.float64) / HD))
    t = np.arange(T, dtype=np.float64)
    freqs = np.outer(t, inv_freq)                      # [T, 64]
    emb = np.concatenate([freqs, freqs], axis=-1)      # [T, 128]
    cos = np.cos(emb).astype(np.float32)
    sin = np.sin(emb).astype(np.float32)
    sin_signed = sin.copy()
    sin_signed[:, :HD // 2] *= -1.0                    # first half gets -sin
    # diagonal-block causal masks, scoresT layout: mask[v][s, t] = (128v+s <= t)
    mask = np.zeros((4, P, TCW), dtype=np.float32)
    s = np.arange(P)[:, None]
    tcol = np.arange(TCW)[None, :]
    for v in range(4):
        mask[v] = (v * P + s <= tcol).astype(np.float32)
    ident = np.eye(P, dtype=np.float32)
    return cos, sin_signed, mask, ident


def kernel(x, w_qkv, w_proj, q_gain):
    global LAST_RESULT
    x = np.asarray(x, dtype=np.float32)
    w_qkv = np.asarray(w_qkv, dtype=np.float32)
    w_proj = np.asarray(w_proj, dtype=np.float32)
    q_gain = np.asarray(q_gain, dtype=np.float32)

    cos, sin_signed, mask, ident = _host_tables()
    nc = _get_nc()

    in_maps = []
    for r in range(8):
        b, g = r // 4, r % 4
        wq = w_qkv[:, g * JQ:(g + 1) * JQ]
        wk = w_qkv[:, C + g * HD:C + (g + 1) * HD]
        wv = w_qkv[:, C + KV_DIM + g * HD:C + KV_DIM + (g + 1) * HD]
        in_maps.append({
            "xb": np.ascontiguousarray(x[b]),
            "wqkv": np.ascontiguousarray(np.concatenate([wq, wk, wv], axis=1)),
            "wproj": np.ascontiguousarray(w_proj[g * JQ:(g + 1) * JQ, :]),
            "gain": np.ascontiguousarray(q_gain[g * REP:(g + 1) * REP].reshape(1, REP)),
            "costab": cos,
            "sintab": sin_signed,
            "maskdiag": mask,
            "ident": ident,
        })

    trace = os.environ.get("KERNEL_TRACE") == "1"
    if trace:
        try:
            import antenv.axon_hooks  # noqa: F401
        except ImportError:
            trace = False
    res = run_bass_kernel_spmd(nc, in_maps, core_ids=list(range(8)), trace=trace)
    LAST_RESULT = res

    out = np.zeros((B, T, C), dtype=np.float32)
    for r in range(8):
        b = r // 4
        out[b] += res.results[r]["y"]
    return out


# revision 12
# speedup vs baseline: 1.0003x; 1.0003x over previous
"""Trainium2 Bass kernel for CausalSelfAttention (GQA + per-head RMS norm + RoPE).

Sharding: 8 cores = batch(2) x kv-head-group(4). Each core computes, for its
(b, g): qkv projection (its 4 rep q heads + 1 kv head), per-head RMS norm,
RoPE, causal attention, and a partial output projection (its 512 rows of
w_proj). Host sums the 4 partial projections per batch element.

Math notes:
  - Token-level rms_norm(x) commutes out of q/k (they are re-normalized per
    head, and rms_norm is scale-invariant), so only v is scaled by the
    per-token rstd(x). This avoids normalizing x before the qkv matmul and
    lets the qkv matmul run on raw x^T (built on-chip via PE transposes).
  - Scores are computed transposed (scoresT[s, t]) so that exp(scoresT) is
    directly the rhs of the attn@v matmul (contraction over s on partitions)
    and the attention output lands as aoT[d, t], which is exactly the lhsT
    the output projection needs. No attention-matrix transposes.
  - Softmax runs without max subtraction: q, k are unit-RMS (|q| <= 1.5*
    sqrt(128)) so scores*scale is bounded by ~17 and exp stays in fp32 range.
  - The softmax denominator is computed by a ones-vector matmul accumulated
    over s-tiles in PSUM, so the exp tiles are only ever consumed by the PE.
  - q_gain and the softmax 1/sqrt(HD) are folded into the per-head rstd and
    the exp scale respectively.
  - Matmul operands are stored as float32r (rounded at the producing
    instruction) for 4x PE throughput vs float32; KERNEL_MMDT=f32 falls back.
"""

import os

import numpy as np

from concourse import bacc, bass, mybir
from concourse import tile
from concourse.bass_utils import run_bass_kernel_spmd

# Problem shape (hardcoded per contract)
B, T, C = 2, 2048, 2048
N_HEADS, N_KV = 16, 4
HD = C // N_HEADS            # 128
REP = N_HEADS // N_KV        # 4
KV_DIM = N_KV * HD           # 512
P = 128
TT = T // P                  # 16 token tiles
KT = C // P                  # 16 contraction tiles
JQ = REP * HD                # 512 local q cols
JTOT = JQ + 2 * HD           # 768 local qkv cols
TCW = 512                    # attention t-chunk width
NTC = T // TCW               # 4
EPS = 1.1920929e-07
SCALE = 1.0 / float(np.sqrt(HD))

F32 = mybir.dt.float32
AF = mybir.ActivationFunctionType


def _emit(nc, mm_f32r=True):
    # MDT: dtype of every matmul operand tile. float32r tiles are written
    # (rounded) by the producing DVE/ACT/DMA instruction and only ever read
    # by the PE.
    MDT = mybir.dt.float32r if mm_f32r else F32

    x_d = nc.dram_tensor("xb", [T, C], F32, kind="ExternalInput")
    wqkv_d = nc.dram_tensor("wqkv", [C, JTOT], MDT, kind="ExternalInput")
    wproj_d = nc.dram_tensor("wproj", [JQ, C], MDT, kind="ExternalInput")
    gain_d = nc.dram_tensor("gain", [1, REP], F32, kind="ExternalInput")
    cos_d = nc.dram_tensor("costab", [T, HD], F32, kind="ExternalInput")
    sin_d = nc.dram_tensor("sintab", [T, HD], F32, kind="ExternalInput")  # [:, :64] = -sin
    mask_d = nc.dram_tensor("maskdiag", [4, P, TCW], F32, kind="ExternalInput")
    id_d = nc.dram_tensor("ident", [P, P], F32, kind="ExternalInput")
    y_d = nc.dram_tensor("y", [T, C], F32, kind="ExternalOutput")

    with tile.TileContext(nc) as tc:
        with tc.tile_pool(name="persist", bufs=1) as pp, \
             tc.tile_pool(name="psum", bufs=1, space="PSUM") as psp:
            # Long-lived activations (matmul operands -> MDT)
            qT = [pp.tile([P, T], MDT, name=f"qT{h}", tag=f"qT{h}") for h in range(REP)]
            kTt = pp.tile([P, T], MDT, name="kTt", tag="kTt")
            vN = pp.tile([P, TT, HD], MDT, name="vN", tag="vN")

            # ---------------- Phase 1: qkv + norms + rope + transposes -------
            with tc.tile_pool(name="ph1", bufs=1) as p1:
                wqkv_sb = p1.tile([P, KT, JTOT], MDT, name="wqkv_sb", tag="wqkv")
                nc.sync.dma_start(
                    out=wqkv_sb,
                    in_=wqkv_d.ap().rearrange("(kt p) j -> p kt j", p=P))
                cos_sb = p1.tile([P, TT, HD], F32, name="cos_sb", tag="cos")
                nc.sync.dma_start(
                    out=cos_sb, in_=cos_d.ap().rearrange("(tt p) d -> p tt d", p=P))
                sin_sb = p1.tile([P, TT, HD], F32, name="sin_sb", tag="sin")
                nc.sync.dma_start(
                    out=sin_sb, in_=sin_d.ap().rearrange("(tt p) d -> p tt d", p=P))
                id_sb = p1.tile([P, P], F32, name="id_sb", tag="ident")
                nc.sync.dma_start(out=id_sb, in_=id_d.ap())
                eps_t = p1.tile([P, 1], F32, name="eps_t", tag="eps")
                nc.vector.memset(eps_t, EPS)

                # broadcast gain [1,4] -> [128,4] via 0-stride DMA replication
                gainb = p1.tile([P, REP], F32, name="gainb", tag="gainb")
                nc.sync.dma_start(out=gainb,
                                  in_=gain_d.ap()[0].partition_broadcast(P))

                for tt in range(TT):
                    x_t = p1.tile([P, C], F32, name=f"x_{tt}", tag="x", bufs=2)
                    nc.sync.dma_start(out=x_t, in_=x_d.ap()[tt * P:(tt + 1) * P, :])

                    # token rstd (for v): 1/sqrt(mean(x^2)+eps)
                    sx4 = p1.tile([P, 4], F32, name=f"sx4_{tt}", tag="sx4", bufs=2)
                    for i in range(4):
                        scr = p1.tile([P, 512], F32, name=f"scrx_{tt}_{i}", tag="scr", bufs=2)
                        nc.scalar.activation(scr, x_t[:, i * 512:(i + 1) * 512],
                                             AF.Square, accum_out=sx4[:, i:i + 1])
                    ssx = p1.tile([P, 1], F32, name=f"ssx_{tt}", tag="ssx", bufs=2)
                    nc.vector.reduce_sum(ssx, sx4, axis=mybir.AxisListType.X)
                    srx = p1.tile([P, 1], F32, name=f"srx_{tt}", tag="srx", bufs=2)
                    nc.scalar.activation(srx, ssx, AF.Sqrt, scale=1.0 / C, bias=eps_t)
                    rstdx = p1.tile([P, 1], F32, name=f"rstdx_{tt}", tag="rstdx", bufs=2)
                    nc.vector.reciprocal(rstdx, srx)

                    # transpose raw x tile -> xT blocks (lhsT for qkv matmul)
                    xTt = p1.tile([P, C], MDT, name=f"xT_{tt}", tag="xT", bufs=3)
                    for grp in range(4):
                        tp = psp.tile([P, 512], F32, name=f"tp_{tt}_{grp}", tag="mm", bufs=4)
                        for j in range(4):
                            kt = grp * 4 + j
                            nc.tensor.transpose(tp[:, j * P:(j + 1) * P],
                                                x_t[:, kt * P:(kt + 1) * P], id_sb)
                        nc.vector.tensor_copy(xTt[:, grp * 512:(grp + 1) * 512], tp)

                    # qkv matmuls: q_ps [P, 512], kv_ps [P, 256]
                    q_ps = psp.tile([P, JQ], F32, name=f"qps_{tt}", tag="mm", bufs=4)
                    kv_ps = psp.tile([P, 2 * HD], F32, name=f"kvps_{tt}", tag="small", bufs=2)
                    for kt in range(KT):
                        lb = xTt[:, kt * P:(kt + 1) * P]
                        nc.tensor.matmul(q_ps, lb, wqkv_sb[:, kt, 0:JQ],
                                         start=(kt == 0), stop=(kt == KT - 1))
                        nc.tensor.matmul(kv_ps, lb, wqkv_sb[:, kt, JQ:JTOT],
                                         start=(kt == 0), stop=(kt == KT - 1))

                    # ---- q: per-head rms norm (x gain) + rope ----
                    ssq4 = p1.tile([P, REP], F32, name=f"ssq4_{tt}", tag="ssq4", bufs=2)
                    for h in range(REP):
                        scr = p1.tile([P, 512], F32, name=f"scrq_{tt}_{h}", tag="scr", bufs=2)
                        nc.scalar.activation(scr[:, :HD], q_ps[:, h * HD:(h + 1) * HD],
                                             AF.Square, accum_out=ssq4[:, h:h + 1])
                    srq = p1.tile([P, REP], F32, name=f"srq_{tt}", tag="srq", bufs=2)
                    nc.scalar.activation(srq, ssq4, AF.Sqrt, scale=1.0 / HD, bias=eps_t)
                    rstdq = p1.tile([P, REP], F32, name=f"rstdq_{tt}", tag="rstdq", bufs=2)
                    nc.vector.reciprocal(rstdq, srq)
                    rstdqg = p1.tile([P, REP], F32, name=f"rstdqg_{tt}", tag="rstdqg", bufs=2)
                    nc.vector.tensor_mul(rstdqg, rstdq, gainb)

                    qn_t = p1.tile([P, JQ], F32, name=f"qn_{tt}", tag="qn", bufs=2)
                    for h in range(REP):
                        nc.vector.tensor_scalar_mul(qn_t[:, h * HD:(h + 1) * HD],
                                                    q_ps[:, h * HD:(h + 1) * HD],
                                                    rstdqg[:, h:h + 1])
                    qn3 = qn_t.rearrange("p (h d) -> p h d", h=REP)
                    cos_t = cos_sb[:, tt, :]
                    sin_t = sin_sb[:, tt, :]
                    qf_t = p1.tile([P, JQ], F32, name=f"qf_{tt}", tag="qf", bufs=2)
                    qf3 = qf_t.rearrange("p (h d) -> p h d", h=REP)
                    qB_t = p1.tile([P, JQ], F32, name=f"qB_{tt}", tag="qB", bufs=2)
                    qB3 = qB_t.rearrange("p (h d) -> p h d", h=REP)
                    H2 = HD // 2
                    nc.vector.tensor_mul(qf3, qn3,
                                         cos_t[:, None, :].broadcast_to([P, REP, HD]))
                    nc.vector.tensor_mul(qB3[:, :, 0:H2], qn3[:, :, H2:HD],
                                         sin_t[:, None, 0:H2].broadcast_to([P, REP, H2]))
                    nc.vector.tensor_mul(qB3[:, :, H2:HD], qn3[:, :, 0:H2],
                                         sin_t[:, None, H2:HD].broadcast_to([P, REP, H2]))
                    nc.vector.tensor_add(qf3, qf3, qB3)

                    # ---- k: rms norm + rope ----
                    ssk = p1.tile([P, 1], F32, name=f"ssk_{tt}", tag="ssk", bufs=2)
                    scrk = p1.tile([P, 512], F32, name=f"scrk_{tt}", tag="scr", bufs=2)
                    nc.scalar.activation(scrk[:, :HD], kv_ps[:, 0:HD], AF.Square,
                                         accum_out=ssk)
                    srk = p1.tile([P, 1], F32, name=f"srk_{tt}", tag="srk", bufs=2)
                    nc.scalar.activation(srk, ssk, AF.Sqrt, scale=1.0 / HD, bias=eps_t)
                    rstdk = p1.tile([P, 1], F32, name=f"rstdk_{tt}", tag="rstdk", bufs=2)
                    nc.vector.reciprocal(rstdk, srk)
                    kn_t = p1.tile([P, HD], F32, name=f"kn_{tt}", tag="kn", bufs=2)
                    nc.vector.tensor_scalar_mul(kn_t, kv_ps[:, 0:HD], rstdk)
                    kf_t = p1.tile([P, HD], F32, name=f"kf_{tt}", tag="kf", bufs=2)
                    kB_t = p1.tile([P, HD], F32, name=f"kB_{tt}", tag="kB", bufs=2)
                    nc.vector.tensor_mul(kf_t, kn_t, cos_t)
                    nc.vector.tensor_mul(kB_t[:, 0:H2], kn_t[:, H2:HD], sin_t[:, 0:H2])
                    nc.vector.tensor_mul(kB_t[:, H2:HD], kn_t[:, 0:H2], sin_t[:, H2:HD])
                    nc.vector.tensor_add(kf_t, kf_t, kB_t)

                    # ---- v: scale rows by token rstd ----
                    nc.vector.tensor_scalar_mul(vN[:, tt, :], kv_ps[:, HD:2 * HD], rstdx)

                    # ---- transpose q heads + k into [d, t] layout ----
                    tq = psp.tile([P, 512], F32, name=f"tq_{tt}", tag="mm", bufs=4)
                    for h in range(REP):
                        nc.tensor.transpose(tq[:, h * P:(h + 1) * P],
                                            qf_t[:, h * P:(h + 1) * P], id_sb)
                    for h in range(REP):
                        nc.vector.tensor_copy(qT[h][:, tt * P:(tt + 1) * P],
                                              tq[:, h * P:(h + 1) * P])
                    tk = psp.tile([P, HD], F32, name=f"tk_{tt}", tag="small", bufs=2)
                    nc.tensor.transpose(tk, kf_t, id_sb)
                    nc.vector.tensor_copy(kTt[:, tt * P:(tt + 1) * P], tk)

            # ---------------- Phase 2+3: attention + projection --------------
            with tc.tile_pool(name="ph23", bufs=1) as p2:
                wproj_sb = p2.tile([P, REP, C], MDT, name="wproj_sb", tag="wproj")
                nc.sync.dma_start(
                    out=wproj_sb,
                    in_=wproj_d.ap().rearrange("(h p) c -> p h c", p=P))
                mask_sb = p2.tile([P, 4, TCW], F32, name="mask_sb", tag="mask")
                nc.sync.dma_start(
                    out=mask_sb, in_=mask_d.ap().rearrange("v p t -> p v t"))
                ones_f = p2.tile([P, P], F32, name="ones_f", tag="ones_f")
                nc.vector.memset(ones_f, 1.0)
                ones_col = p2.tile([P, 1], MDT, name="ones_col", tag="ones_col")
                nc.vector.tensor_copy(ones_col, ones_f[:, 0:1])
                ones_row = p2.tile([1, P], MDT, name="ones_row", tag="ones_row")
                nc.vector.tensor_copy(ones_row, ones_f[0:1, :])

                for tci in range(NTC):
                    ao = []
                    for h in range(REP):
                        o_ps = psp.tile([P, TCW], F32, name=f"ops_{tci}_{h}",
                                        tag="acc", bufs=2)
                        denom = p2.tile([P, TCW], MDT, name=f"dn_{tci}_{h}",
                                        tag="denom", bufs=2)
                        nst = 4 * (tci + 1)
                        for st in range(nst):
                            sc = psp.tile([P, TCW], F32, name=f"sc_{tci}_{h}_{st}",
                                          tag="mm", bufs=4)
                            nc.tensor.matmul(sc, kTt[:, st * P:(st + 1) * P],
                                             qT[h][:, tci * TCW:(tci + 1) * TCW],
                                             start=True, stop=True)
                            et = p2.tile([P, TCW], MDT, name=f"et_{tci}_{h}_{st}",
                                         tag="et", bufs=6)
                            dv = st - 4 * tci
                            if dv >= 0:
                                er = p2.tile([P, TCW], F32, name=f"er_{tci}_{h}_{st}",
                                             tag="er", bufs=2)
                                nc.scalar.activation(er, sc, AF.Exp, scale=SCALE)
                                nc.vector.tensor_mul(et, er, mask_sb[:, dv, :])
                            else:
                                nc.scalar.activation(et, sc, AF.Exp, scale=SCALE)
                            # denominator partial sums on DVE (partition-wise)
                            if st == 0:
                                nc.vector.tensor_copy(denom, et)
                            else:
                                nc.vector.tensor_add(denom, denom, et)
                            nc.tensor.matmul(o_ps, vN[:, st, :], et,
                                             start=(st == 0), stop=(st == nst - 1))
                        # partition-sum of denom, broadcast, then wide reciprocal
                        ds_ps = psp.tile([1, TCW], F32, name=f"ds_{tci}_{h}",
                                         tag="small", bufs=2)
                        nc.tensor.matmul(ds_ps, ones_col, denom, start=True, stop=True)
                        dsum = p2.tile([1, TCW], MDT, name=f"dsum_{tci}_{h}",
                                       tag="dsum", bufs=2)
                        with nc.allow_low_precision(reason="fp32r matmul operand"):
                            nc.vector.tensor_copy(dsum, ds_ps)
                        rb_ps = psp.tile([P, TCW], F32, name=f"rb_{tci}_{h}",
                                         tag="mm", bufs=4)
                        nc.tensor.matmul(rb_ps, ones_row, dsum, start=True, stop=True)
                        rb = p2.tile([P, TCW], F32, name=f"rbs_{tci}_{h}",
                                     tag="rb", bufs=2)
                        nc.vector.reciprocal(rb, rb_ps)
                        aot = p2.tile([P, TCW], MDT, name=f"ao_{tci}_{h}",
                                      tag="ao", bufs=8)
                        nc.vector.tensor_mul(aot, o_ps, rb)
                        ao.append(aot)

                    # projection for this t-chunk
                    for ttl in range(4):
                        yt = p2.tile([P, C], F32, name=f"y_{tci}_{ttl}", tag="y", bufs=2)
                        for ncs in range(4):
                            y_ps = psp.tile([P, 512], F32, name=f"yps_{tci}_{ttl}_{ncs}",
                                            tag="mm", bufs=4)
                            for h in range(REP):
                                nc.tensor.matmul(
                                    y_ps, ao[h][:, ttl * P:(ttl + 1) * P],
                                    wproj_sb[:, h, ncs * 512:(ncs + 1) * 512],
                                    start=(h == 0), stop=(h == REP - 1))
                            nc.vector.tensor_copy(yt[:, ncs * 512:(ncs + 1) * 512], y_ps)
                        row = (tci * 4 + ttl) * P
                        nc.sync.dma_start(out=y_d.ap()[row:row + P, :], in_=yt)

    return nc


_NC_CACHE = {}
LAST_RESULT = None


def _get_nc():
    key = os.environ.get("KERNEL_MMDT", "f32r")
    if key not in _NC_CACHE:
        nc = bacc.Bacc("TRN2", target_bir_lowering=False, debug=False)
        _emit(nc, mm_f32r=(key != "f32"))
        nc.compile()
        _NC_CACHE[key] = nc
    return _NC_CACHE[key]


def _host_tables():
    inv_freq = 1.0 / (10000.0 ** (np.arange(0, HD, 2, dtype=np.float64) / HD))
    t = np.arange(T, dtype=np.float64)
    freqs = np.outer(t, inv_freq)                      # [T, 64]
    emb = np.concatenate([freqs, freqs], axis=-1)      # [T, 128]
    cos = np.cos(emb).astype(np.float32)
    sin = np.sin(emb).astype(np.float32)
    sin_signed = sin.copy()
    sin_signed[:, :HD // 2] *= -1.0                    # first half gets -sin
    # diagonal-block causal masks, scoresT layout: mask[v][s, t] = (128v+s <= t)
    mask = np.zeros((4, P, TCW), dtype=np.float32)
    s = np.arange(P)[:, None]
    tcol = np.arange(TCW)[None, :]
    for v in range(4):
        mask[v] = (v * P + s <= tcol).astype(np.float32)
    ident = np.eye(P, dtype=np.float32)
    return cos, sin_signed, mask, ident


def kernel(x, w_qkv, w_proj, q_gain):
    global LAST_RESULT
    x = np.asarray(x, dtype=np.float32)
    w_qkv = np.asarray(w_qkv, dtype=np.float32)
    w_proj = np.asarray(w_proj, dtype=np.float32)
    q_gain = np.asarray(q_gain, dtype=np.float32)

    cos, sin_signed, mask, ident = _host_tables()
    nc = _get_nc()

    in_maps = []
    for r in range(8):
        b, g = r // 4, r % 4
        wq = w_qkv[:, g * JQ:(g + 1) * JQ]
        wk = w_qkv[:, C + g * HD:C + (g + 1) * HD]
        wv = w_qkv[:, C + KV_DIM + g * HD:C + KV_DIM + (g + 1) * HD]
        in_maps.append({
            "xb": np.ascontiguousarray(x[b]),
            "wqkv": np.ascontiguousarray(np.concatenate([wq, wk, wv], axis=1)),
            "wproj": np.ascontiguousarray(w_proj[g * JQ:(g + 1) * JQ, :]),
            "gain": np.ascontiguousarray(q_gain[g * REP:(g + 1) * REP].reshape(1, REP)),
            "costab": cos,
            "sintab": sin_signed,
            "maskdiag": mask,
            "ident": ident,
        })

    trace = os.environ.get("KERNEL_TRACE") == "1"
    if trace:
        try:
            import antenv.axon_hooks  # noqa: F401
        except ImportError:
            trace = False
    res = run_bass_kernel_spmd(nc, in_maps, core_ids=list(range(8)), trace=trace)
    LAST_RESULT = res

    out = np.zeros((B, T, C), dtype=np.float32)
    for r in range(8):
        b = r // 4
        out[b] += res.results[r]["y"]
    return out


# revision 13
# speedup vs baseline: 1.3980x; 1.3976x over previous
"""Trainium2 Bass kernel for CausalSelfAttention (GQA + per-head RMS norm + RoPE).

Sharding: 8 cores = batch(2) x kv-head-group(4). Each core computes, for its
(b, g): qkv projection (its 4 rep q heads + 1 kv head), per-head RMS norm,
RoPE, causal attention, and a partial output projection (its 512 rows of
w_proj). Host sums the 4 partial projections per batch element.

Math notes:
  - Token-level rms_norm(x) commutes out of q/k (they are re-normalized per
    head, and rms_norm is scale-invariant), so only v is scaled by the
    per-token rstd(x). This avoids normalizing x before the qkv matmul and
    lets the qkv matmul run on raw x^T (built on-chip via PE transposes).
  - Scores are computed transposed (scoresT[s, t]) so that exp(scoresT) is
    directly the rhs of the attn@v matmul (contraction over s on partitions)
    and the attention output lands as aoT[d, t], which is exactly the lhsT
    the output projection needs. No attention-matrix transposes.
  - Softmax runs without max subtraction: q, k are unit-RMS (|q| <= 1.5*
    sqrt(128)) so scores*scale is bounded by ~17 and exp stays in fp32 range.
  - Causal masking adds -1e30 into the diagonal score tiles (PSUM, DVE) so
    exp maps masked entries to exactly 0.
  - The softmax denominator accumulates on DVE in f32 (reading the exp tiles
    bitcast to f32 for the 2x DVE mode); only the final add rounds to f32r.
  - Attention processes head PAIRS with interleaved score/attnV matmuls, and
    the previous chunk's projection matmuls are woven into the attention
    stream, keeping the PE dense so the HAM clock stays at 2.4 GHz.
  - Matmul operands are stored as float32r (rounded at the producing
    instruction): the PE streams them at ~2x the float32 rate.
"""

import os

import numpy as np

from concourse import bacc, bass, mybir
from concourse import tile
from concourse.bass_utils import run_bass_kernel_spmd

# Problem shape (hardcoded per contract)
B, T, C = 2, 2048, 2048
N_HEADS, N_KV = 16, 4
HD = C // N_HEADS            # 128
REP = N_HEADS // N_KV        # 4
KV_DIM = N_KV * HD           # 512
P = 128
TT = T // P                  # 16 token tiles
KT = C // P                  # 16 contraction tiles
JQ = REP * HD                # 512 local q cols
JTOT = JQ + 2 * HD           # 768 local qkv cols
TCW = 512                    # attention t-chunk width
NTC = T // TCW               # 4
EPS = 1.1920929e-07
SCALE = 1.0 / float(np.sqrt(HD))
MASKVAL = -1.0e30

F32 = mybir.dt.float32
AF = mybir.ActivationFunctionType


def _emit(nc, mm_f32r=True):
    # MDT: dtype of every matmul operand tile. float32r tiles are written
    # (rounded) by the producing DVE/ACT/DMA instruction and only ever read
    # by the PE (except via value-preserving bitcasts back to f32).
    MDT = mybir.dt.float32r if mm_f32r else F32

    x_d = nc.dram_tensor("xb", [T, C], F32, kind="ExternalInput")
    wqkv_d = nc.dram_tensor("wqkv", [C, JTOT], MDT, kind="ExternalInput")
    wproj_d = nc.dram_tensor("wproj", [JQ, C], MDT, kind="ExternalInput")
    gain_d = nc.dram_tensor("gain", [1, REP], F32, kind="ExternalInput")
    cos_d = nc.dram_tensor("costab", [T, HD], F32, kind="ExternalInput")
    sin_d = nc.dram_tensor("sintab", [T, HD], F32, kind="ExternalInput")  # [:, :64] = -sin
    mask_d = nc.dram_tensor("maskdiag", [4, P, TCW], F32, kind="ExternalInput")  # 0 / -1e30
    id_d = nc.dram_tensor("ident", [P, P], F32, kind="ExternalInput")
    y_d = nc.dram_tensor("y", [T, C], F32, kind="ExternalOutput")

    with tile.TileContext(nc) as tc:
        with tc.tile_pool(name="persist", bufs=1) as pp, \
             tc.tile_pool(name="psum", bufs=1, space="PSUM") as psp:
            # Long-lived activations (matmul operands -> MDT)
            qT = [pp.tile([P, T], MDT, name=f"qT{h}", tag=f"qT{h}") for h in range(REP)]
            kTt = pp.tile([P, T], MDT, name="kTt", tag="kTt")
            vN = pp.tile([P, TT, HD], MDT, name="vN", tag="vN")
            # Attention-phase small tiles live in the persistent pool so the
            # attention stream never waits on the phase-1 pool release.
            mask_sb = pp.tile([P, 4, TCW], F32, name="mask_sb", tag="mask")
            nc.sync.dma_start(out=mask_sb,
                              in_=mask_d.ap().rearrange("v p t -> p v t"))
            ones_f = pp.tile([P, 1], F32, name="ones_f", tag="ones_f")
            nc.vector.memset(ones_f, 1.0)
            ones_col = pp.tile([P, 1], MDT, name="ones_col", tag="ones_col")
            nc.vector.tensor_copy(ones_col, ones_f)
            onesr_f = pp.tile([1, P], F32, name="onesr_f", tag="onesr_f")
            nc.vector.memset(onesr_f, 1.0)
            ones_row = pp.tile([1, P], MDT, name="ones_row", tag="ones_row")
            nc.vector.tensor_copy(ones_row, onesr_f)

            # ---------------- Phase 1: qkv + norms + rope + transposes -------
            with tc.tile_pool(name="ph1", bufs=1) as p1:
                wqkv_sb = p1.tile([P, KT, JTOT], MDT, name="wqkv_sb", tag="wqkv")
                wq4 = wqkv_d.ap().rearrange("(kg kt p) j -> p kg kt j", p=P, kg=4)
                for kg in range(4):
                    nc.sync.dma_start(out=wqkv_sb[:, kg * 4:(kg + 1) * 4, :],
                                      in_=wq4[:, kg])
                id_sb = p1.tile([P, P], F32, name="id_sb", tag="ident")
                nc.sync.dma_start(out=id_sb, in_=id_d.ap())
                eps_t = p1.tile([P, 1], F32, name="eps_t", tag="eps")
                nc.vector.memset(eps_t, EPS)

                # broadcast gain [1,4] -> [128,4] via 0-stride DMA replication
                gainb = p1.tile([P, REP], F32, name="gainb", tag="gainb")
                nc.sync.dma_start(out=gainb,
                                  in_=gain_d.ap()[0].partition_broadcast(P))

                cos4 = cos_d.ap().rearrange("(tt p) d -> p tt d", p=P)
                sin4 = sin_d.ap().rearrange("(tt p) d -> p tt d", p=P)

                prev = None  # software-pipelined q/k transposes
                H2 = HD // 2

                def emit_qk_transposes(qf_t, kf_t, ptt):
                    tq = psp.tile([P, 512], F32, name=f"tq_{ptt}", tag="mm", bufs=4)
                    for h in range(REP):
                        nc.tensor.transpose(tq[:, h * P:(h + 1) * P],
                                            qf_t[:, h * P:(h + 1) * P], id_sb)
                    for h in range(REP):
                        nc.vector.tensor_copy(qT[h][:, ptt * P:(ptt + 1) * P],
                                              tq[:, h * P:(h + 1) * P])
                    tk = psp.tile([P, HD], F32, name=f"tk_{ptt}", tag="small", bufs=2)
                    nc.tensor.transpose(tk, kf_t, id_sb)
                    nc.vector.tensor_copy(kTt[:, ptt * P:(ptt + 1) * P], tk)

                for tt in range(TT):
                    x_t = p1.tile([P, C], F32, name=f"x_{tt}", tag="x", bufs=3)
                    nc.sync.dma_start(out=x_t, in_=x_d.ap()[tt * P:(tt + 1) * P, :])
                    cos_t = p1.tile([P, HD], F32, name=f"cos_{tt}", tag="cos", bufs=2)
                    nc.sync.dma_start(out=cos_t, in_=cos4[:, tt])
                    sin_t = p1.tile([P, HD], F32, name=f"sin_{tt}", tag="sin", bufs=2)
                    nc.sync.dma_start(out=sin_t, in_=sin4[:, tt])

                    # token rstd (for v): 1/sqrt(mean(x^2)+eps)
                    sx4 = p1.tile([P, 4], F32, name=f"sx4_{tt}", tag="sx4", bufs=2)
                    for i in range(4):
                        scr = p1.tile([P, 512], F32, name=f"scrx_{tt}_{i}", tag="scr", bufs=2)
                        nc.scalar.activation(scr, x_t[:, i * 512:(i + 1) * 512],
                                             AF.Square, accum_out=sx4[:, i:i + 1])
                    ssx = p1.tile([P, 1], F32, name=f"ssx_{tt}", tag="ssx", bufs=2)
                    nc.vector.reduce_sum(ssx, sx4, axis=mybir.AxisListType.X)
                    srx = p1.tile([P, 1], F32, name=f"srx_{tt}", tag="srx", bufs=2)
                    nc.scalar.activation(srx, ssx, AF.Sqrt, scale=1.0 / C, bias=eps_t)
                    rstdx = p1.tile([P, 1], F32, name=f"rstdx_{tt}", tag="rstdx", bufs=2)
                    nc.vector.reciprocal(rstdx, srx)

                    # transpose raw x tile -> xT blocks (lhsT for qkv matmul)
                    xTt = p1.tile([P, C], MDT, name=f"xT_{tt}", tag="xT", bufs=2)
                    for grp in range(4):
                        tp = psp.tile([P, 512], F32, name=f"tp_{tt}_{grp}", tag="mm", bufs=4)
                        for j in range(4):
                            kt = grp * 4 + j
                            nc.tensor.transpose(tp[:, j * P:(j + 1) * P],
                                                x_t[:, kt * P:(kt + 1) * P], id_sb)
                        nc.vector.tensor_copy(xTt[:, grp * 512:(grp + 1) * 512], tp)

                    # qkv matmuls: q_ps [P, 512], kv_ps [P, 256]
                    q_ps = psp.tile([P, JQ], F32, name=f"qps_{tt}", tag="mm", bufs=4)
                    kv_ps = psp.tile([P, 2 * HD], F32, name=f"kvps_{tt}", tag="small", bufs=2)
                    for kt in range(KT):
                        lb = xTt[:, kt * P:(kt + 1) * P]
                        nc.tensor.matmul(q_ps, lb, wqkv_sb[:, kt, 0:JQ],
                                         start=(kt == 0), stop=(kt == KT - 1))
                        nc.tensor.matmul(kv_ps, lb, wqkv_sb[:, kt, JQ:JTOT],
                                         start=(kt == 0), stop=(kt == KT - 1))

                    # previous tile's q/k transposes: their rope finished while
                    # this tile's qkv matmuls ran, so the PE never waits on DVE
                    if prev is not None:
                        emit_qk_transposes(*prev)

                    # ---- q: per-head rms norm (x gain) + rope ----
                    ssq4 = p1.tile([P, REP], F32, name=f"ssq4_{tt}", tag="ssq4", bufs=2)
                    for h in range(REP):
                        scr = p1.tile([P, 512], F32, name=f"scrq_{tt}_{h}", tag="scr", bufs=2)
                        nc.scalar.activation(scr[:, :HD], q_ps[:, h * HD:(h + 1) * HD],
                                             AF.Square, accum_out=ssq4[:, h:h + 1])
                    srq = p1.tile([P, REP], F32, name=f"srq_{tt}", tag="srq", bufs=2)
                    nc.scalar.activation(srq, ssq4, AF.Sqrt, scale=1.0 / HD, bias=eps_t)
                    rstdq = p1.tile([P, REP], F32, name=f"rstdq_{tt}", tag="rstdq", bufs=2)
                    nc.vector.reciprocal(rstdq, srq)
                    rstdqg = p1.tile([P, REP], F32, name=f"rstdqg_{tt}", tag="rstdqg", bufs=2)
                    nc.vector.tensor_mul(rstdqg, rstdq, gainb)

                    qn_t = p1.tile([P, JQ], F32, name=f"qn_{tt}", tag="qn", bufs=2)
                    for h in range(REP):
                        nc.vector.tensor_scalar_mul(qn_t[:, h * HD:(h + 1) * HD],
                                                    q_ps[:, h * HD:(h + 1) * HD],
                                                    rstdqg[:, h:h + 1])
                    qn3 = qn_t.rearrange("p (h d) -> p h d", h=REP)
                    qf_t = p1.tile([P, JQ], F32, name=f"qf_{tt}", tag="qf", bufs=2)
                    qf3 = qf_t.rearrange("p (h d) -> p h d", h=REP)
                    qB_t = p1.tile([P, JQ], F32, name=f"qB_{tt}", tag="qB", bufs=2)
                    qB3 = qB_t.rearrange("p (h d) -> p h d", h=REP)
                    nc.vector.tensor_mul(qf3, qn3,
                                         cos_t[:, None, :].broadcast_to([P, REP, HD]))
                    nc.vector.tensor_mul(qB3[:, :, 0:H2], qn3[:, :, H2:HD],
                                         sin_t[:, None, 0:H2].broadcast_to([P, REP, H2]))
                    nc.vector.tensor_mul(qB3[:, :, H2:HD], qn3[:, :, 0:H2],
                                         sin_t[:, None, H2:HD].broadcast_to([P, REP, H2]))
                    nc.vector.tensor_add(qf3, qf3, qB3)

                    # ---- k: rms norm + rope ----
                    ssk = p1.tile([P, 1], F32, name=f"ssk_{tt}", tag="ssk", bufs=2)
                    scrk = p1.tile([P, 512], F32, name=f"scrk_{tt}", tag="scr", bufs=2)
                    nc.scalar.activation(scrk[:, :HD], kv_ps[:, 0:HD], AF.Square,
                                         accum_out=ssk)
                    srk = p1.tile([P, 1], F32, name=f"srk_{tt}", tag="srk", bufs=2)
                    nc.scalar.activation(srk, ssk, AF.Sqrt, scale=1.0 / HD, bias=eps_t)
                    rstdk = p1.tile([P, 1], F32, name=f"rstdk_{tt}", tag="rstdk", bufs=2)
                    nc.vector.reciprocal(rstdk, srk)
                    kn_t = p1.tile([P, HD], F32, name=f"kn_{tt}", tag="kn", bufs=2)
                    nc.vector.tensor_scalar_mul(kn_t, kv_ps[:, 0:HD], rstdk)
                    kf_t = p1.tile([P, HD], F32, name=f"kf_{tt}", tag="kf", bufs=2)
                    kB_t = p1.tile([P, HD], F32, name=f"kB_{tt}", tag="kB", bufs=2)
                    nc.vector.tensor_mul(kf_t, kn_t, cos_t)
                    nc.vector.tensor_mul(kB_t[:, 0:H2], kn_t[:, H2:HD], sin_t[:, 0:H2])
                    nc.vector.tensor_mul(kB_t[:, H2:HD], kn_t[:, 0:H2], sin_t[:, H2:HD])
                    nc.vector.tensor_add(kf_t, kf_t, kB_t)

                    # ---- v: scale rows by token rstd ----
                    nc.vector.tensor_scalar_mul(vN[:, tt, :], kv_ps[:, HD:2 * HD], rstdx)

                    prev = (qf_t, kf_t, tt)

                emit_qk_transposes(*prev)

            # ---------------- Phase 2+3: attention + projection --------------
            with tc.tile_pool(name="ph23", bufs=1) as p2:
                wproj_sb = p2.tile([P, REP, C], MDT, name="wproj_sb", tag="wproj")
                wp4 = wproj_d.ap().rearrange("(h p) c -> p h c", p=P)
                for h in range(REP):
                    nc.sync.dma_start(out=wproj_sb[:, h:h + 1, :], in_=wp4[:, h:h + 1, :])

                def attend(tci, h, o_ps):
                    """Yields once per (score, attnV) step; caller interleaves."""
                    nst = 4 * (tci + 1)
                    denf = p2.tile([P, TCW], F32, name=f"dnf_{tci}_{h}",
                                   tag="denf", bufs=4)
                    den_r = None
                    for st in range(nst):
                        sc = psp.tile([P, TCW], F32, name=f"sc_{tci}_{h}_{st}",
                                      tag="mm", bufs=4)
                        nc.tensor.matmul(sc, kTt[:, st * P:(st + 1) * P],
                                         qT[h][:, tci * TCW:(tci + 1) * TCW],
                                         start=True, stop=True)
                        dv = st - 4 * tci
                        if dv >= 0:  # diagonal: add -1e30 mask into PSUM
                            nc.vector.tensor_add(sc, sc, mask_sb[:, dv, :])
                        et = p2.tile([P, TCW], MDT, name=f"et_{tci}_{h}_{st}",
                                     tag="et", bufs=8)
                        nc.scalar.activation(et, sc, AF.Exp, scale=SCALE)
                        etf = et.bitcast(F32)
                        if st == 0:
                            nc.vector.tensor_copy(denf, etf)
                        elif st < nst - 1:
                            nc.vector.tensor_add(denf, denf, etf)
                        else:
                            den_r = p2.tile([P, TCW], MDT, name=f"dnr_{tci}_{h}",
                                            tag="denr", bufs=4)
                            nc.vector.tensor_add(den_r, denf, etf)
                        nc.tensor.matmul(o_ps, vN[:, st, :], et,
                                         start=(st == 0), stop=(st == nst - 1))
                        yield
                    # tail: denom partition-sum, broadcast, reciprocal, rescale
                    ds_ps = psp.tile([1, TCW], F32, name=f"ds_{tci}_{h}",
                                     tag="small", bufs=2)
                    nc.tensor.matmul(ds_ps, ones_col, den_r, start=True, stop=True)
                    dsum = p2.tile([1, TCW], MDT, name=f"dsum_{tci}_{h}",
                                   tag="dsum", bufs=4)
                    with nc.allow_low_precision(reason="fp32r matmul operand"):
                        nc.vector.tensor_copy(dsum, ds_ps)
                    rb_ps = psp.tile([P, TCW], F32, name=f"rb_{tci}_{h}",
                                     tag="mm", bufs=4)
                    nc.tensor.matmul(rb_ps, ones_row, dsum, start=True, stop=True)
                    rb = p2.tile([P, TCW], F32, name=f"rbs_{tci}_{h}",
                                 tag="rb", bufs=4)
                    nc.vector.reciprocal(rb, rb_ps)
                    aot = p2.tile([P, TCW], MDT, name=f"ao_{tci}_{h}",
                                  tag="ao", bufs=8)
                    nc.vector.tensor_mul(aot, o_ps, rb)
                    ao_tiles[(tci, h)] = aot
                    yield

                def proj_steps(tci):
                    """Projection for chunk tci as small emit-steps (4 MMs each)."""
                    for ttl in range(4):
                        yt = p2.tile([P, C], F32, name=f"y_{tci}_{ttl}", tag="y", bufs=2)
                        for ncs in range(4):
                            def step(tci=tci, ttl=ttl, ncs=ncs, yt=yt):
                                y_ps = psp.tile([P, 512], F32,
                                                name=f"yps_{tci}_{ttl}_{ncs}",
                                                tag="small", bufs=2)
                                for h in range(REP):
                                    nc.tensor.matmul(
                                        y_ps,
                                        ao_tiles[(tci, h)][:, ttl * P:(ttl + 1) * P],
                                        wproj_sb[:, h, ncs * 512:(ncs + 1) * 512],
                                        start=(h == 0), stop=(h == REP - 1))
                                nc.vector.tensor_copy(yt[:, ncs * 512:(ncs + 1) * 512],
                                                      y_ps)
                            yield step
                        def dma_step(tci=tci, ttl=ttl, yt=yt):
                            row = (tci * 4 + ttl) * P
                            nc.sync.dma_start(out=y_d.ap()[row:row + P, :], in_=yt)
                        yield dma_step

                ao_tiles = {}
                pending = iter(())
                for tci in range(NTC):
                    for hp in (0, 2):
                        o0 = psp.tile([P, TCW], F32, name=f"ops_{tci}_{hp}",
                                      tag="acc", bufs=2)
                        o1 = psp.tile([P, TCW], F32, name=f"ops_{tci}_{hp + 1}",
                                      tag="acc", bufs=2)
                        g0 = attend(tci, hp, o0)
                        g1 = attend(tci, hp + 1, o1)
                        alive = True
                        while alive:
                            alive = False
                            for g in (g0, g1):
                                try:
                                    next(g)
                                    alive = True
                                except StopIteration:
                                    pass
                            # weave one projection step of the previous chunk
                            s = next(pending, None)
                            if s is not None:
                                s()
                    # flush any remaining projection steps of the previous chunk
                    for s in pending:
                        s()
                    pending = iter(list(proj_steps(tci)))
                for s in pending:
                    s()

    return nc


_NC_CACHE = {}
LAST_RESULT = None


def _get_nc():
    key = os.environ.get("KERNEL_MMDT", "f32r")
    if key not in _NC_CACHE:
        nc = bacc.Bacc("TRN2", target_bir_lowering=False, debug=False)
        _emit(nc, mm_f32r=(key != "f32"))
        nc.compile()
        _NC_CACHE[key] = nc
    return _NC_CACHE[key]


def _host_tables():
    inv_freq = 1.0 / (10000.0 ** (np.arange(0, HD, 2, dtype=np.float64) / HD))
    t = np.arange(T, dtype=np.float64)
    freqs = np.outer(t, inv_freq)                      # [T, 64]
    emb = np.concatenate([freqs, freqs], axis=-1)      # [T, 128]
    cos = np.cos(emb).astype(np.float32)
    sin = np.sin(emb).astype(np.float32)
    sin_signed = sin.copy()
    sin_signed[:, :HD // 2] *= -1.0                    # first half gets -sin
    # diagonal-block causal mask biases, scoresT layout:
    # mask[v][s, t] = 0 if (128v+s <= t) else -1e30
    mask = np.zeros((4, P, TCW), dtype=np.float32)
    s = np.arange(P)[:, None]
    tcol = np.arange(TCW)[None, :]
    for v in range(4):
        mask[v] = np.where(v * P + s <= tcol, 0.0, MASKVAL).astype(np.float32)
    ident = np.eye(P, dtype=np.float32)
    return cos, sin_signed, mask, ident


def kernel(x, w_qkv, w_proj, q_gain):
    global LAST_RESULT
    x = np.asarray(x, dtype=np.float32)
    w_qkv = np.asarray(w_qkv, dtype=np.float32)
    w_proj = np.asarray(w_proj, dtype=np.float32)
    q_gain = np.asarray(q_gain, dtype=np.float32)

    cos, sin_signed, mask, ident = _host_tables()
    nc = _get_nc()

    in_maps = []
    for r in range(8):
        b, g = r // 4, r % 4
        wq = w_qkv[:, g * JQ:(g + 1) * JQ]
        wk = w_qkv[:, C + g * HD:C + (g + 1) * HD]
        wv = w_qkv[:, C + KV_DIM + g * HD:C + KV_DIM + (g + 1) * HD]
        in_maps.append({
            "xb": np.ascontiguousarray(x[b]),
            "wqkv": np.ascontiguousarray(np.concatenate([wq, wk, wv], axis=1)),
            "wproj": np.ascontiguousarray(w_proj[g * JQ:(g + 1) * JQ, :]),
            "gain": np.ascontiguousarray(q_gain[g * REP:(g + 1) * REP].reshape(1, REP)),
            "costab": cos,
            "sintab": sin_signed,
            "maskdiag": mask,
            "ident": ident,
        })

    trace = os.environ.get("KERNEL_TRACE") == "1"
    if trace:
        try:
            import antenv.axon_hooks  # noqa: F401
        except ImportError:
            trace = False
    res = run_bass_kernel_spmd(nc, in_maps, core_ids=list(range(8)), trace=trace)
    LAST_RESULT = res

    out = np.zeros((B, T, C), dtype=np.float32)
    for r in range(8):
        b = r // 4
        out[b] += res.results[r]["y"]
    return out
